# revision 10
# baseline (speedup 1.0000x reference)
"""Trainium2 Bass kernel for nn_CAM_Module (channel attention).

Reference computation (per batch b):
    att = q[b] @ k[b].T          # (C, C)
    out = att @ v[b] + v1[b]     # (C, N)

We use associativity to avoid materializing the (C, C) matrix:
    out[b] = q[b] @ (k[b].T @ v[b]) + v1[b]
where s = k.T @ v is only (N, N) = (49, 49). This reduces FLOPs by ~21x
and turns the problem memory-bound.

Sharding: pure data parallel — batch dim (128) split across 8 cores,
16 batches per core, no cross-core communication.

Per-core layout trick: channels are tiled c = 8*p + t (p = SBUF
partition, t = free-dim tile index). This makes every DMA contiguous
per partition (1568B runs) while keeping the contraction over c on the
partition axis for the tensor engine.

Batches are processed in PAIRS packed into single matmuls:
  - step 1: lhsT = [kA | kB] (128 x 98), rhs = [vA | vB] -> s_pair (98 x 98),
    whose diagonal 49x49 blocks are s_A and s_B (off-diagonal is unused).
  - transposes: q tiles of both batches transposed together (128 x 98 in).
  - step 2: one matmul per c-tile: lhsT = qT_pair (98 x 128),
    rhs = block-diagonal [s_A 0; 0 s_B] (98 x 98) -> out tile (128 x 98),
    columns 0-48 = batch A, 49-97 = batch B.
"""

import os

os.environ.setdefault("JAX_PLATFORMS", "axon")

import numpy as np

B, C, H, W = 128, 1024, 7, 7
N = H * W  # 49
NCORES = 8
BPC = B // NCORES  # 16 batches per core
P = 128  # SBUF partitions
T = C // P  # 8 c-tiles, c = T*p + t
PAIRS = BPC // 2

_NC_CACHE = {}


def _build_nc():
    import concourse.mybir as mybir
    import concourse.tile as tile
    from concourse import bacc
    from concourse.masks import make_identity

    f32 = mybir.dt.float32
    nc = bacc.Bacc("TRN2", target_bir_lowering=False, debug=False)

    NN = 2 * N  # 98

    # inputs are host-side pre-tiled to [pair, p, t, a, n] so that each
    # DMA is a contiguous identity copy AND each matmul slice [:, t, :, :]
    # has a single contiguous free dimension (a, n) = 98 wide.
    vd = nc.dram_tensor("v1", [PAIRS, P, T, 2, N], f32, kind="ExternalInput").ap()
    qd = nc.dram_tensor("q1", [PAIRS, P, T, 2, N], f32, kind="ExternalInput").ap()
    kd = nc.dram_tensor("k1", [PAIRS, P, T, 2, N], f32, kind="ExternalInput").ap()
    md = nc.dram_tensor("m0", [NN, NN], f32, kind="ExternalInput").ap()
    od = nc.dram_tensor("out0", [BPC, P, T, N], f32, kind="ExternalOutput").ap()

    with tile.TileContext(nc) as tc:
        with (
            tc.tile_pool(name="const", bufs=1) as cpool,
            tc.tile_pool(name="io", bufs=3) as iop,
            tc.tile_pool(name="qt", bufs=2) as qtp,
            tc.tile_pool(name="ssb", bufs=2) as sbp,
            tc.tile_pool(name="osb", bufs=4) as outp,
            tc.tile_pool(name="ps_s", bufs=2, space="PSUM") as pss,
            tc.tile_pool(name="ps_qt", bufs=1, space="PSUM") as psq,
            tc.tile_pool(name="ps_o", bufs=2, space="PSUM") as pso,
        ):
            ident = cpool.tile([P, P], f32)
            make_identity(nc, ident[:])
            # block-diagonal 0/1 mask selecting the per-batch diagonal
            # blocks of the packed s_pair matrix
            mask = cpool.tile([NN, NN], f32)
            nc.sync.dma_start(out=mask[:], in_=md[:])

            for i in range(PAIRS):
                a = 2 * i
                kt = iop.tile([P, T, 2, N], f32, tag="k")
                vt = iop.tile([P, T, 2, N], f32, tag="v")
                qt = iop.tile([P, T, 2, N], f32, tag="q")
                nc.sync.dma_start(out=kt[:], in_=kd[i])
                nc.sync.dma_start(out=vt[:], in_=vd[i])
                nc.sync.dma_start(out=qt[:], in_=qd[i])

                # step 1: s_pair = [kA|kB].T @ [vA|vB], accumulated over c-tiles
                s_ps = pss.tile([NN, NN], f32)
                for t in range(T):
                    nc.tensor.matmul(
                        s_ps[:],
                        kt[:, t, :, :],
                        vt[:, t, :, :],
                        start=(t == 0),
                        stop=(t == T - 1),
                    )

                # transpose q tiles: [128, 98] -> [98, 128] per c-tile
                qT_ps = psq.tile([NN, T, P], f32)
                for t in range(T):
                    nc.tensor.transpose(qT_ps[:, t, :], qt[:, t, :, :], ident[:])

                qT_sb = qtp.tile([NN, T, P], f32)
                nc.scalar.copy(out=qT_sb[:], in_=qT_ps[:])

                # block-diagonal s in SBUF: mask away the cross-batch blocks
                s_sb = sbp.tile([NN, NN], f32)
                nc.vector.tensor_mul(out=s_sb[:], in0=s_ps[:], in1=mask[:])

                # step 2: out tile t = qT_pair[t].T @ s_blockdiag
                o_ps = pso.tile([P, T, P], f32)
                for t in range(T):
                    nc.tensor.matmul(
                        o_ps[:, t, 0:NN],
                        qT_sb[:, t, :],
                        s_sb[:],
                        start=True,
                        stop=True,
                    )

                # residual add + store, per batch in the pair
                for w in range(2):
                    o_sb = outp.tile([P, T, N], f32, tag="osb")
                    nc.vector.tensor_add(
                        out=o_sb[:],
                        in0=o_ps[:, :, w * N : (w + 1) * N],
                        in1=vt[:, :, w, :],
                    )
                    nc.sync.dma_start(out=od[a + w], in_=o_sb[:])

    nc.compile()
    return nc


def _get_nc():
    if "nc" not in _NC_CACHE:
        _NC_CACHE["nc"] = _build_nc()
    return _NC_CACHE["nc"]


def _shard(x):
    # (B, C, H, W) -> per-core (NCORES, PAIRS, P, T, 2, N), c = T*p + t,
    # with the two batches of each pair interleaved innermost so every
    # DMA is contiguous and matmul slices have one free dim.
    x = np.asarray(x, dtype=np.float32).reshape(NCORES, PAIRS, 2, P, T, N)
    return np.ascontiguousarray(x.transpose(0, 1, 3, 4, 2, 5))


def _blockdiag_mask():
    m = np.zeros((2 * N, 2 * N), dtype=np.float32)
    m[:N, :N] = 1.0
    m[N:, N:] = 1.0
    return m


def kernel(v1, q1, k1):
    nc = _get_nc()
    from concourse.bass_utils import run_bass_kernel_spmd

    v = _shard(v1)
    q = _shard(q1)
    k = _shard(k1)
    m = _blockdiag_mask()
    in_maps = [{"v1": v[i], "q1": q[i], "k1": k[i], "m0": m} for i in range(NCORES)]
    res = run_bass_kernel_spmd(nc, in_maps, list(range(NCORES))).results
    out = np.stack([res[i]["out0"] for i in range(NCORES)])
    return out.reshape(B, C, H, W)


def estimate_time_ns():
    """Cost-model timing of the per-core program (TimelineSim)."""
    from concourse.timeline_sim import TimelineSim

    nc = _get_nc()
    sim = TimelineSim(nc)
    sim.simulate()
    return sim.time


# revision 22
# speedup vs baseline: 1.3283x; 1.3283x over previous
"""Trainium2 Bass kernel for nn_CAM_Module (channel attention).

Reference computation (per batch b):
    att = q[b] @ k[b].T          # (C, C)
    out = att @ v[b] + v1[b]     # (C, N)

We use associativity to avoid materializing the (C, C) matrix:
    out[b] = q[b] @ (k[b].T @ v[b]) + v1[b]
where s = k.T @ v is only (N, N) = (49, 49). This reduces FLOPs by ~21x
and makes the problem memory-bound (~12.9 MB of HBM traffic per core).

Sharding: pure data parallel — batch dim (128) split across 8 cores,
16 batches per core, no cross-core communication.

Per-core layout: channels are tiled c = 8*p + t (p = SBUF partition,
t = free-dim tile index), and batches are interleaved in PAIRS on the
host so that all DMAs are contiguous identity copies and every matmul
operand slice has a single contiguous free dimension:

  - step 1: lhsT = [kA|kB] (128 x 98), rhs = [vA|vB] -> s_pair (98 x 98)
    accumulated over the 8 c-tiles; its diagonal 49x49 blocks are s_A
    and s_B (off-diagonal blocks are cross-batch garbage).
  - mask:   s_sbuf = s_pair * blockdiag_mask (zeroes the cross blocks).
  - transposes: q tiles of both batches transposed together
    ([128, 98] -> [98, 128] per c-tile, PE transpose via identity).
  - step 2: one matmul per c-tile: lhsT = qT_pair (98 x 128),
    rhs = block-diag s (98 x 98) -> out tile (128 x 98), columns 0-48 =
    batch A, 49-97 = batch B.
  - one fused residual add per pair (+v1) and one contiguous store.
"""

import os

os.environ.setdefault("JAX_PLATFORMS", "axon")

import numpy as np

B, C, H, W = 128, 1024, 7, 7
N = H * W  # 49
NCORES = 8
BPC = B // NCORES  # 16 batches per core
P = 128  # SBUF partitions
T = C // P  # 8 c-tiles, c = T*p + t
PAIRS = BPC // 2

_NC_CACHE = {}

# tunables (overridable for TimelineSim sweeps)
CFG = {
    "io_bufs": 3,
    "qt_bufs": 2,
    "ssb_bufs": 2,
    "osb_bufs": 3,
    "ps_s_bufs": 2,
    "ps_qt_bufs": 1,
    "ps_o_bufs": 2,
    "qt_copy_split": 2,  # chunks for the qT PSUM->SBUF copy
    "dma_group": 2,  # pairs per input DMA
    "out_on_scalar": True,  # issue store DMAs on the ACT HWDGE ring
    # bf16 for the q/s path: fp32 matmul costs 4 cycles/row on the PE
    # (two half-speed passes); casting step-2's operands to bf16 runs it
    # and the q transposes at full rate. Step 1 (k.T @ v) stays fp32, so
    # s is exact; only the final 49-term contraction sees bf16 rounding.
    "q_bf16": True,
    # also cast k/v to bf16 during the load DMA: step 1 runs at full PE
    # rate too (s accumulates in fp32 PSUM regardless)
    "kv_bf16": True,
}


def _build_nc():
    import concourse.mybir as mybir
    import concourse.tile as tile
    from concourse import bacc
    from concourse.masks import make_identity

    f32 = mybir.dt.float32
    bf16 = mybir.dt.bfloat16
    qdt = bf16 if CFG["q_bf16"] else f32
    nc = bacc.Bacc("TRN2", target_bir_lowering=False, debug=False)

    NN = 2 * N  # 98
    G = CFG["dma_group"]
    assert PAIRS % G == 0

    # all tensors are host-side pre-tiled to [pair, p, t, a, n] so that
    # every DMA is a contiguous identity copy AND each matmul slice
    # [:, t, :, :] has a single contiguous free dimension (a, n) = 98.
    vd = nc.dram_tensor("v1", [PAIRS, P, T, 2, N], f32, kind="ExternalInput").ap()
    qd = nc.dram_tensor("q1", [PAIRS, P, T, 2, N], f32, kind="ExternalInput").ap()
    kd = nc.dram_tensor("k1", [PAIRS, P, T, 2, N], f32, kind="ExternalInput").ap()
    md = nc.dram_tensor("m0", [NN, NN], f32, kind="ExternalInput").ap()
    od = nc.dram_tensor("out0", [PAIRS, P, T, 2, N], f32, kind="ExternalOutput").ap()

    with tile.TileContext(nc) as tc:
        with (
            tc.tile_pool(name="const", bufs=1) as cpool,
            tc.tile_pool(name="io", bufs=CFG["io_bufs"]) as iop,
            tc.tile_pool(name="qt", bufs=CFG["qt_bufs"]) as qtp,
            tc.tile_pool(name="ssb", bufs=CFG["ssb_bufs"]) as sbp,
            tc.tile_pool(name="osb", bufs=CFG["osb_bufs"]) as outp,
            tc.tile_pool(name="ps_s", bufs=CFG["ps_s_bufs"], space="PSUM") as pss,
            tc.tile_pool(name="ps_qt", bufs=CFG["ps_qt_bufs"], space="PSUM") as psq,
            tc.tile_pool(name="ps_o", bufs=CFG["ps_o_bufs"], space="PSUM") as pso,
        ):
            ident = cpool.tile([P, P], qdt)
            make_identity(nc, ident[:])
            # block-diagonal 0/1 mask selecting the per-batch diagonal
            # blocks of the packed s_pair matrix
            mask = cpool.tile([NN, NN], f32)
            nc.sync.dma_start(out=mask[:], in_=md[:])

            out_dma = nc.scalar if CFG["out_on_scalar"] else nc.sync

            for gi in range(PAIRS // G):
                kvdt = bf16 if CFG["kv_bf16"] else f32
                kt = iop.tile([P, G, T, 2, N], kvdt, tag="k")
                vt = iop.tile([P, G, T, 2, N], kvdt, tag="v")
                qt = iop.tile([P, G, T, 2, N], qdt, tag="q")
                # casting DMA (fp32 -> bf16) must go through SWDGE (gpsimd)
                q_dma = nc.gpsimd if CFG["q_bf16"] else nc.sync
                kv_dma = nc.gpsimd if CFG["kv_bf16"] else nc.sync
                sl = slice(gi * G, (gi + 1) * G)
                if G == 1:
                    kv_dma.dma_start(out=kt[:, 0], in_=kd[gi * G])
                    kv_dma.dma_start(out=vt[:, 0], in_=vd[gi * G])
                    q_dma.dma_start(out=qt[:, 0], in_=qd[gi * G])
                else:
                    kv_dma.dma_start(
                        out=kt[:], in_=kd[sl].rearrange("g p t a n -> p g t a n")
                    )
                    kv_dma.dma_start(
                        out=vt[:], in_=vd[sl].rearrange("g p t a n -> p g t a n")
                    )
                    q_dma.dma_start(
                        out=qt[:], in_=qd[sl].rearrange("g p t a n -> p g t a n")
                    )

                for g in range(G):
                    i = gi * G + g
                    # step 1: s_pair = [kA|kB].T @ [vA|vB] over c-tiles
                    s_ps = pss.tile([NN, NN], f32)
                    for t in range(T):
                        nc.tensor.matmul(
                            s_ps[:],
                            kt[:, g, t, :, :],
                            vt[:, g, t, :, :],
                            start=(t == 0),
                            stop=(t == T - 1),
                        )

                    # transpose q tiles: [128, 98] -> [98, 128] per c-tile
                    qT_ps = psq.tile([NN, T, P], qdt)
                    for t in range(T):
                        nc.tensor.transpose(
                            qT_ps[:, t, :], qt[:, g, t, :, :], ident[:]
                        )

                    qT_sb = qtp.tile([NN, T, P], qdt)
                    nch = CFG["qt_copy_split"]
                    tw = T // nch
                    for cc in range(nch):
                        nc.scalar.copy(
                            out=qT_sb[:, cc * tw : (cc + 1) * tw, :],
                            in_=qT_ps[:, cc * tw : (cc + 1) * tw, :],
                        )

                    # block-diagonal s in SBUF: mask the cross-batch blocks
                    # (cast to the step-2 matmul dtype on the way out)
                    s_sb = sbp.tile([NN, NN], qdt)
                    nc.vector.tensor_mul(out=s_sb[:], in0=s_ps[:], in1=mask[:])

                    # step 2: out tile t = qT_pair[t].T @ s_blockdiag
                    o_ps = pso.tile([P, T, P], f32)
                    for t in range(T):
                        nc.tensor.matmul(
                            o_ps[:, t, 0:NN],
                            qT_sb[:, t, :],
                            s_sb[:],
                            start=True,
                            stop=True,
                        )

                    # fused residual add for the whole pair + one store
                    o_sb = outp.tile([P, T, 2, N], f32, tag="osb")
                    nc.vector.tensor_add(
                        out=o_sb[:],
                        in0=o_ps[:, :, 0:NN],
                        in1=vt[:, g],
                    )
                    out_dma.dma_start(out=od[i], in_=o_sb[:])

    nc.compile()
    return nc


def _get_nc():
    if "nc" not in _NC_CACHE:
        _NC_CACHE["nc"] = _build_nc()
    return _NC_CACHE["nc"]


def _shard(x):
    # (B, C, H, W) -> per-core (NCORES, PAIRS, P, T, 2, N), c = T*p + t,
    # with the two batches of each pair interleaved innermost so every
    # DMA is contiguous and matmul slices have one free dim.
    x = np.asarray(x, dtype=np.float32).reshape(NCORES, PAIRS, 2, P, T, N)
    return np.ascontiguousarray(x.transpose(0, 1, 3, 4, 2, 5))


def _blockdiag_mask():
    m = np.zeros((2 * N, 2 * N), dtype=np.float32)
    m[:N, :N] = 1.0
    m[N:, N:] = 1.0
    return m


def kernel(v1, q1, k1):
    nc = _get_nc()
    from concourse.bass_utils import run_bass_kernel_spmd

    v = _shard(v1)
    q = _shard(q1)
    k = _shard(k1)
    m = _blockdiag_mask()
    in_maps = [{"v1": v[i], "q1": q[i], "k1": k[i], "m0": m} for i in range(NCORES)]
    res = run_bass_kernel_spmd(nc, in_maps, list(range(NCORES))).results
    out = np.stack([res[i]["out0"] for i in range(NCORES)])
    # (NCORES, PAIRS, P, T, 2, N) -> (B, C, H, W)
    out = out.transpose(0, 1, 4, 2, 3, 5).reshape(B, C, H, W)
    return np.ascontiguousarray(out)


def estimate_time_ns():
    """Cost-model timing of the per-core program (TimelineSim)."""
    from concourse.timeline_sim import TimelineSim

    nc = _get_nc()
    sim = TimelineSim(nc)
    sim.simulate()
    return sim.time


# revision 23
# speedup vs baseline: 1.7595x; 1.3246x over previous
"""Trainium2 Bass kernel for nn_CAM_Module (channel attention).

Reference computation (per batch b):
    att = q[b] @ k[b].T          # (C, C)
    out = att @ v[b] + v1[b]     # (C, N)

We use associativity to avoid materializing the (C, C) matrix:
    out[b] = q[b] @ (k[b].T @ v[b]) + v1[b]
where s = k.T @ v is only (N, N) = (49, 49). This reduces FLOPs by ~21x
and makes the problem memory-bound (~12.9 MB of HBM traffic per core).

Sharding: pure data parallel — batch dim (128) split across 8 cores,
16 batches per core, no cross-core communication.

Per-core layout: channels are tiled c = 8*p + t (p = SBUF partition,
t = free-dim tile index), and batches are interleaved in PAIRS on the
host so that all DMAs are contiguous identity copies and every matmul
operand slice has a single contiguous free dimension:

  - step 1: lhsT = [kA|kB] (128 x 98), rhs = [vA|vB] -> s_pair (98 x 98)
    accumulated over the 8 c-tiles; its diagonal 49x49 blocks are s_A
    and s_B (off-diagonal blocks are cross-batch garbage).
  - mask:   s_sbuf = s_pair * blockdiag_mask (zeroes the cross blocks).
  - transposes: q tiles of both batches transposed together
    ([128, 98] -> [98, 128] per c-tile, PE transpose via identity).
  - step 2: one matmul per c-tile: lhsT = qT_pair (98 x 128),
    rhs = block-diag s (98 x 98) -> out tile (128 x 98), columns 0-48 =
    batch A, 49-97 = batch B.
  - one fused residual add per pair (+v1) and one contiguous store.
"""

import os

os.environ.setdefault("JAX_PLATFORMS", "axon")

import numpy as np

B, C, H, W = 128, 1024, 7, 7
N = H * W  # 49
NCORES = 8
BPC = B // NCORES  # 16 batches per core
P = 128  # SBUF partitions
T = C // P  # 8 c-tiles, c = T*p + t
PAIRS = BPC // 2

_NC_CACHE = {}

# tunables (overridable for TimelineSim sweeps)
CFG = {
    "io_bufs": 4,
    "qt_bufs": 2,
    "ssb_bufs": 2,
    "osb_bufs": 3,
    "ps_s_bufs": 2,
    "ps_qt_bufs": 2,
    "ps_o_bufs": 2,
    "qt_copy_split": 1,  # chunks for the qT PSUM->SBUF copy
    "dma_group": 2,  # pairs per input DMA
    "out_on_scalar": False,  # issue store DMAs on the ACT HWDGE ring
    # bf16 for the q/s path: fp32 matmul costs 4 cycles/row on the PE
    # (two half-speed passes); casting step-2's operands to bf16 runs it
    # and the q transposes at full rate. Step 1 (k.T @ v) stays fp32, so
    # s is exact; only the final 49-term contraction sees bf16 rounding.
    "q_bf16": True,
    # also cast k/v to bf16 during the load DMA: step 1 runs at full PE
    # rate too (s accumulates in fp32 PSUM regardless)
    "kv_bf16": True,
}


def _build_nc():
    import concourse.mybir as mybir
    import concourse.tile as tile
    from concourse import bacc
    from concourse.masks import make_identity

    f32 = mybir.dt.float32
    bf16 = mybir.dt.bfloat16
    qdt = bf16 if CFG["q_bf16"] else f32
    nc = bacc.Bacc("TRN2", target_bir_lowering=False, debug=False)

    NN = 2 * N  # 98
    G = CFG["dma_group"]
    assert PAIRS % G == 0

    # all tensors are host-side pre-tiled to [pair, p, t, a, n] so that
    # every DMA is a contiguous identity copy AND each matmul slice
    # [:, t, :, :] has a single contiguous free dimension (a, n) = 98.
    vd = nc.dram_tensor("v1", [PAIRS, P, T, 2, N], f32, kind="ExternalInput").ap()
    qd = nc.dram_tensor("q1", [PAIRS, P, T, 2, N], f32, kind="ExternalInput").ap()
    kd = nc.dram_tensor("k1", [PAIRS, P, T, 2, N], f32, kind="ExternalInput").ap()
    md = nc.dram_tensor("m0", [NN, NN], f32, kind="ExternalInput").ap()
    od = nc.dram_tensor("out0", [PAIRS, P, T, 2, N], f32, kind="ExternalOutput").ap()

    with tile.TileContext(nc) as tc:
        with (
            tc.tile_pool(name="const", bufs=1) as cpool,
            tc.tile_pool(name="io", bufs=CFG["io_bufs"]) as iop,
            tc.tile_pool(name="qt", bufs=CFG["qt_bufs"]) as qtp,
            tc.tile_pool(name="ssb", bufs=CFG["ssb_bufs"]) as sbp,
            tc.tile_pool(name="osb", bufs=CFG["osb_bufs"]) as outp,
            tc.tile_pool(name="ps_s", bufs=CFG["ps_s_bufs"], space="PSUM") as pss,
            tc.tile_pool(name="ps_qt", bufs=CFG["ps_qt_bufs"], space="PSUM") as psq,
            tc.tile_pool(name="ps_o", bufs=CFG["ps_o_bufs"], space="PSUM") as pso,
        ):
            ident = cpool.tile([P, P], qdt)
            make_identity(nc, ident[:])
            # block-diagonal 0/1 mask selecting the per-batch diagonal
            # blocks of the packed s_pair matrix
            mask = cpool.tile([NN, NN], f32)
            nc.sync.dma_start(out=mask[:], in_=md[:])

            out_dma = nc.scalar if CFG["out_on_scalar"] else nc.sync

            for gi in range(PAIRS // G):
                kvdt = bf16 if CFG["kv_bf16"] else f32
                kt = iop.tile([P, G, T, 2, N], kvdt, tag="k")
                vt = iop.tile([P, G, T, 2, N], kvdt, tag="v")
                qt = iop.tile([P, G, T, 2, N], qdt, tag="q")
                # casting DMA (fp32 -> bf16) must go through SWDGE (gpsimd)
                q_dma = nc.gpsimd if CFG["q_bf16"] else nc.sync
                kv_dma = nc.gpsimd if CFG["kv_bf16"] else nc.sync
                sl = slice(gi * G, (gi + 1) * G)
                if G == 1:
                    kv_dma.dma_start(out=kt[:, 0], in_=kd[gi * G])
                    kv_dma.dma_start(out=vt[:, 0], in_=vd[gi * G])
                    q_dma.dma_start(out=qt[:, 0], in_=qd[gi * G])
                else:
                    kv_dma.dma_start(
                        out=kt[:], in_=kd[sl].rearrange("g p t a n -> p g t a n")
                    )
                    kv_dma.dma_start(
                        out=vt[:], in_=vd[sl].rearrange("g p t a n -> p g t a n")
                    )
                    q_dma.dma_start(
                        out=qt[:], in_=qd[sl].rearrange("g p t a n -> p g t a n")
                    )

                for g in range(G):
                    i = gi * G + g
                    # step 1: s_pair = [kA|kB].T @ [vA|vB] over c-tiles
                    s_ps = pss.tile([NN, NN], f32)
                    for t in range(T):
                        nc.tensor.matmul(
                            s_ps[:],
                            kt[:, g, t, :, :],
                            vt[:, g, t, :, :],
                            start=(t == 0),
                            stop=(t == T - 1),
                        )

                    # transpose q tiles: [128, 98] -> [98, 128] per c-tile
                    qT_ps = psq.tile([NN, T, P], qdt)
                    for t in range(T):
                        nc.tensor.transpose(
                            qT_ps[:, t, :], qt[:, g, t, :, :], ident[:]
                        )

                    qT_sb = qtp.tile([NN, T, P], qdt)
                    nch = CFG["qt_copy_split"]
                    tw = T // nch
                    for cc in range(nch):
                        nc.scalar.copy(
                            out=qT_sb[:, cc * tw : (cc + 1) * tw, :],
                            in_=qT_ps[:, cc * tw : (cc + 1) * tw, :],
                        )

                    # block-diagonal s in SBUF: mask the cross-batch blocks
                    # (cast to the step-2 matmul dtype on the way out)
                    s_sb = sbp.tile([NN, NN], qdt)
                    nc.vector.tensor_mul(out=s_sb[:], in0=s_ps[:], in1=mask[:])

                    # step 2: out tile t = qT_pair[t].T @ s_blockdiag
                    o_ps = pso.tile([P, T, P], f32)
                    for t in range(T):
                        nc.tensor.matmul(
                            o_ps[:, t, 0:NN],
                            qT_sb[:, t, :],
                            s_sb[:],
                            start=True,
                            stop=True,
                        )

                    # fused residual add for the whole pair + one store
                    o_sb = outp.tile([P, T, 2, N], f32, tag="osb")
                    nc.vector.tensor_add(
                        out=o_sb[:],
                        in0=o_ps[:, :, 0:NN],
                        in1=vt[:, g],
                    )
                    out_dma.dma_start(out=od[i], in_=o_sb[:])

    nc.compile()
    return nc


def _get_nc():
    if "nc" not in _NC_CACHE:
        _NC_CACHE["nc"] = _build_nc()
    return _NC_CACHE["nc"]


def _shard(x):
    # (B, C, H, W) -> per-core (NCORES, PAIRS, P, T, 2, N), c = T*p + t,
    # with the two batches of each pair interleaved innermost so every
    # DMA is contiguous and matmul slices have one free dim.
    x = np.asarray(x, dtype=np.float32).reshape(NCORES, PAIRS, 2, P, T, N)
    return np.ascontiguousarray(x.transpose(0, 1, 3, 4, 2, 5))


def _blockdiag_mask():
    m = np.zeros((2 * N, 2 * N), dtype=np.float32)
    m[:N, :N] = 1.0
    m[N:, N:] = 1.0
    return m


def kernel(v1, q1, k1):
    nc = _get_nc()
    from concourse.bass_utils import run_bass_kernel_spmd

    v = _shard(v1)
    q = _shard(q1)
    k = _shard(k1)
    m = _blockdiag_mask()
    in_maps = [{"v1": v[i], "q1": q[i], "k1": k[i], "m0": m} for i in range(NCORES)]
    res = run_bass_kernel_spmd(nc, in_maps, list(range(NCORES))).results
    out = np.stack([res[i]["out0"] for i in range(NCORES)])
    # (NCORES, PAIRS, P, T, 2, N) -> (B, C, H, W)
    out = out.transpose(0, 1, 4, 2, 3, 5).reshape(B, C, H, W)
    return np.ascontiguousarray(out)


def estimate_time_ns():
    """Cost-model timing of the per-core program (TimelineSim)."""
    from concourse.timeline_sim import TimelineSim

    nc = _get_nc()
    sim = TimelineSim(nc)
    sim.simulate()
    return sim.time
